# revision 1
# baseline (speedup 1.0000x reference)
"""Multi-head attention (B=2, S=2048, D=1024, H=16) on 8 trn2 NeuronCores.

Tensor-parallel over heads (2 heads per core, column-sliced wq/wk/wv) for the
QKV projections and attention; a per-(batch, head-group) AllToAll then
redistributes the attention output so each core computes the output
projection for its own interleaved 512-row slice of the flattened (B*S)
sequence (Megatron-style TP with a sequence-parallel output projection).

Layout/engine choices (timeline-profiled to 225.5us on the v2 cost model,
from a 237.1us starting point):
  - the host supplies x.T and pre-swizzled w tiles so every DMA row is >=
    1KB contiguous (the HWDGE descriptor engine costs 0.62us per DMA, so
    few/large transfers matter); no activation transposes on device
  - x streams in np-major [128,1024] chunks; QKV runs all three projections
    per nb-pair so PE consumption matches the x DMA feed rate -- any PE
    stall resets the tensor engine's p-state ramp to half clock
  - logits are computed transposed [t, s] so the softmax exp (over t) feeds
    the P@V matmul directly -- no probability-matrix transposes
  - ones-columns appended to V produce the softmax denominators in the same
    PV matmul (PSUM rows 64..127), replicated across partitions for a cheap
    vector normalize
  - matmuls run in float32r (full-rate relaxed fp32); the x/w stream and the
    projection tail (attnT, collective buffers, wo) are float16
  - exp runs on ACT from 2x[128,1024] double-buffered PSUM logit tiles; ACT
    paces attention at ~1.04us/tile vs PE's 0.85us, and the tile scheduler
    back-fills PE's slack with the batch-1 QKV stream (tile_wait_until pins
    keep the compile-time scheduler from ordering that stream ahead of
    attention, whose x arrives later than the scheduler's DMA model thinks)
  - attention's 6 PSUM banks are placed on banks whose phase-A tenants die
    early; the 2 QKV-half1 banks are recycled for the output projection so
    its matmuls are not WAR-blocked behind attention's last PSUM reads
  - the four 0.25MB AllToAlls (15us constant + 40GB/s each, serialized on
    the collective engine) overlap attention; only the last is exposed.
    Batch-0's output projection plus a stream of dependency-free warm-up
    matmuls bridge that window so the p-state ramp is still hot when
    batch-1's projection runs behind the final collective; the final
    normalize reads PSUM directly and ships as a 128KB slice-pair, and
    batch-1's projection ends in single-mc groups to shorten the closing
    bias-add + out-DMA chain
"""

import sys

sys.path.insert(0, "/opt/trn_rl_repo")

import numpy as np

import concourse.mybir as mybir
import concourse.tile as tile
from concourse import bacc
from concourse.bass_utils import run_bass_kernel_spmd
from concourse.masks import make_identity

B, S, D = 2, 2048, 1024
H, HD = 16, 64
NCORES = 8
DL = D // NCORES          # 128 local attn dims (2 heads) per core
R = B * S                 # 4096 flattened rows
RSL = R // NCORES         # 512 output rows per core
P = 128
KC = D // P               # 8 contraction chunks of 128
TC = S // P               # 16 key/t chunks per batch
SB = 512                  # moving-operand (N) tile
NSB = (R // 2) // SB      # 4 row-chunks per half
F32 = mybir.dt.float32
F32R = mybir.dt.float32r
F16 = mybir.dt.float16

_CACHE = {}


def _build(n_iters=1, phases=3, bench=False):
    nc = bacc.Bacc("TRN2", target_bir_lowering=False, debug=False,
                   num_devices=NCORES)
    Exp = mybir.ActivationFunctionType.Exp

    kind = "Internal" if bench else "ExternalInput"
    xT = nc.dram_tensor("xT", [D, R], F16, kind=kind)
    # w*S are pre-swizzled on host to the SBUF tile layout [P, KC*P]
    wqS = nc.dram_tensor("wqS", [P, D], F16, kind=kind)
    wkS = nc.dram_tensor("wkS", [P, D], F16, kind=kind)
    wvS = nc.dram_tensor("wvS", [P, D], F16, kind=kind)
    woT = nc.dram_tensor("woT", [D, D], F16, kind=kind)
    bqkv = nc.dram_tensor("bqkv", [DL, 3], F32, kind=kind)
    bo_t = nc.dram_tensor("bo_t", [P, NCORES], F32, kind=kind)
    out = nc.dram_tensor("out", [D, RSL], F32, kind="ExternalOutput")

    with tile.TileContext(nc) as tc:
        with (
            tc.tile_pool(name="const", bufs=1) as const,
            tc.tile_pool(name="persist", bufs=1) as persist,
            tc.tile_pool(name="dram", bufs=1, space="DRAM") as dram,
        ):
            # ---- constants / weights resident in SBUF ----
            w_s = []
            for name, wt in (("wk", wkS), ("wq", wqS), ("wv", wvS)):
                t = const.tile([P, D], F16, tag=f"w_{name}", name=f"w_{name}")
                if bench:
                    nc.vector.memset(t[:], 0.0)
                else:
                    nc.sync.dma_start(t[:], wt[:, :])
                w_s.append(t)
            w_k, w_q, w_v = w_s

            ident = const.tile([P, P], F16, tag="ident")
            make_identity(nc, ident[:])
            bias3 = const.tile([DL, 3], F32, tag="bias3")
            bo_s = const.tile([P, NCORES], F32, tag="bo_s")
            if bench:
                nc.vector.memset(bias3[:], 0.0)
                nc.vector.memset(bo_s[:], 0.0)
            else:
                nc.sync.dma_start(bias3[:], bqkv[:])
                nc.sync.dma_start(bo_s[:], bo_t[:])
            wo_s = [const.tile([P, D], F16, tag=f"wo{kc}", name=f"wo{kc}")
                    for kc in range(KC)]

            # persistent activations
            QT = persist.tile([P, R], F32R, tag="QT")   # [2 heads*64, B*S]
            KT = persist.tile([P, R], F32R, tag="KT")
            VT = persist.tile([P, R], F16, tag="VT")
            # V natural per 128-row t-chunk: [v_h0 |ones| v_h1 |ones]
            vn = persist.tile([P, (R // P) * 256], F16, tag="vn")
            vn3 = vn[:].rearrange("p (g two c) -> p g two c", two=2, c=128)
            nc.gpsimd.memset(vn3[:, :, :, 64:128], 1.0)
            attnT = persist.tile([P, R], F16, tag="attnT")

            # QKV issue order: K first (logits sweep every t-chunk, so K has
            # the earliest deadline), then Q for the first s-half, V, Q rest
            QKV_ORDER = ([(0, nb) for nb in range(NSB)]          # K
                         + [(1, 0), (1, 1)]                      # Q sh0
                         + [(2, nb) for nb in range(NSB)]        # V
                         + [(1, 2), (1, 3)])                     # Q sh1
            W_OF = {0: w_k, 1: w_q, 2: w_v}
            DST_OF = {0: KT, 1: QT, 2: VT}
            BIAS_COL = {0: 1, 1: 0, 2: 2}   # bias3 columns are (q, k, v)

            for it in range(n_iters):
                SH = S // 2
                CW = RSL // 2
                a2a_in = [[dram.tile([NCORES, HD, CW], F16,
                                     tag=f"a2a_in{it}_{b}_{h}",
                                     name=f"a2a_in{it}_{b}_{h}")
                           for h in range(2)] for b in range(B)]
                a2a_out = [[dram.tile([NCORES, HD, CW], F16,
                                      tag=f"a2a_out{it}_{b}_{h}",
                                      name=f"a2a_out{it}_{b}_{h}")
                            for h in range(2)] for b in range(B)]

                def load_half(half, xt_pool):
                    # np-major [128,1024] chunks: the first QKV round is
                    # DMA-complete after ~2MB, and DMA count stays low (the
                    # HWDGE descriptor engine costs 0.62us per DMA)
                    hof = half * (R // 2)
                    xts = {}
                    for np_ in range(2):
                        for kc in range(KC):
                            t = xt_pool.tile([P, 2 * SB], F16, tag="xt",
                                             name=f"xt_{it}_{half}_{np_}_{kc}")
                            nc.sync.dma_start(
                                t[:], xT[kc * P:(kc + 1) * P,
                                         hof + np_ * 2 * SB:
                                         hof + (np_ + 1) * 2 * SB])
                            for i in range(2):
                                xts[(kc, np_ * 2 + i)] = t[:, i * SB:
                                                           (i + 1) * SB]
                    return xts

                def qkv_group(pj, nb, hof, xts, pool, tag, eng):
                    t = pool.tile([P, SB], F32, tag=tag,
                                  name=f"{tag}_{it}_{hof}_{pj}_{nb}")
                    for kc in range(KC):
                        nc.tensor.matmul(
                            t[:], W_OF[pj][:, kc * P:(kc + 1) * P],
                            xts[(kc, nb)],
                            start=(kc == 0), stop=(kc == KC - 1))
                    bc = BIAS_COL[pj]
                    eng.tensor_scalar_add(
                        DST_OF[pj][:, hof + nb * SB:hof + (nb + 1) * SB],
                        t[:], bias3[:, bc:bc + 1])

                def vnat(half, pool, tag):
                    # V natural (+ ones) tiles for this half's t-chunks
                    for g in range(half * 16, half * 16 + 16):
                        pt = pool.tile([P, P], F16, tag=tag,
                                       name=f"pt_{it}_{half}_{g}")
                        nc.tensor.transpose(pt[:], VT[:, g * P:(g + 1) * P],
                                            ident[:])
                        o = g * 256
                        nc.vector.tensor_copy(vn[:, o:o + 64], pt[:, 0:64])
                        nc.vector.tensor_copy(vn[:, o + 128:o + 192],
                                              pt[:, 64:128])

                def attention_batch(b, ps3, exps, norm):
                    base = b * S
                    lg_pre = []
                    if b == 0:
                        # fix the lg tag's two ring slots on banks 0-3
                        # (ps1's early-freed slots) before pv claims them
                        lg_pre = [ps3.tile([P, SH], F32, tag="lg", bufs=2,
                                           name=f"lg_pre_{it}_{k}")
                                  for k in range(2)]
                    for h in range(2):
                        hr = slice(h * HD, (h + 1) * HD)
                        for sh in range(2):
                            sof = base + sh * SH
                            pv = ps3.tile([P, SH], F32, tag="pv", bufs=1,
                                          name=f"pv_{it}_{b}_{h}_{sh}")
                            for tcn in range(TC):
                                ex = exps.tile([P, SH], F16, tag="ex",
                                               name=f"ex_{it}_{b}_{h}_{sh}_{tcn}")
                                lg = (lg_pre.pop(0) if lg_pre else
                                      ps3.tile([P, SH], F32, tag="lg",
                                               bufs=2,
                                               name=f"lg_{it}_{b}_{h}_{sh}_{tcn}"))
                                for sb in range(2):
                                    nc.tensor.matmul(
                                        lg[:, sb * SB:(sb + 1) * SB],
                                        KT[hr, base + tcn * P:
                                           base + (tcn + 1) * P],
                                        QT[hr, sof + sb * SB:
                                           sof + (sb + 1) * SB],
                                        start=True, stop=True)
                                nc.scalar.activation(ex[:], lg[:], Exp,
                                                     scale=1.0 / 8.0)
                                o = (b * TC + tcn) * 256 + h * 128
                                for sb in range(2):
                                    nc.tensor.matmul(
                                        pv[:, sb * SB:(sb + 1) * SB],
                                        vn[:, o:o + 128],
                                        ex[:, sb * SB:(sb + 1) * SB],
                                        start=(tcn == 0), stop=(tcn == TC - 1))
                            if (b, h, sh) == (1, 1, 1):
                                # final chunk: normalize straight out of
                                # PSUM (the bank is never reused) -- the
                                # shortest chain to the last AllToAll
                                rc = norm.tile([HD, SH], F32, tag="rcf")
                                nc.vector.reciprocal(rc[:], pv[64:128, :])
                                nc.vector.tensor_mul(
                                    attnT[h * HD:(h + 1) * HD,
                                          sof:sof + SH],
                                    pv[0:64, :], rc[:])
                            else:
                                vcp = norm.tile([P, SH], F32, tag="vcp")
                                nc.vector.tensor_copy(vcp[:], pv[:])
                                rc = norm.tile([HD, SH], F32, tag="rc")
                                nc.vector.reciprocal(rc[:], vcp[64:128, :])
                                nc.vector.tensor_mul(
                                    attnT[h * HD:(h + 1) * HD,
                                          sof:sof + SH],
                                    vcp[0:64, :], rc[:])
                            # ship the finished half-row-block right away:
                            # the final a2a then waits only on a 128KB DMA
                            if phases >= 3:
                                nc.sync.dma_start(
                                    a2a_in[b][h][4 * sh:4 * sh + 4]
                                    .rearrange("j p c -> p j c"),
                                    attnT[h * HD:(h + 1) * HD,
                                          sof:sof + SH].rearrange(
                                              "p (j c) -> p j c", c=CW))
                        if phases >= 3:
                            nc.gpsimd.collective_compute(
                                "AllToAll", mybir.AluOpType.bypass,
                                replica_groups=[list(range(NCORES))],
                                ins=[a2a_in[b][h].opt()],
                                outs=[a2a_out[b][h].opt()])

                def gather_rh(b, proj):
                    rh_b = proj.tile([P, KC * CW], F16, tag=f"rh{it}_{b}",
                                     name=f"rh{it}_{b}")
                    for h in range(2):
                        # batch-1 h1 lands last: gather it in two halves so
                        # the projection can start on the first four
                        # kc-blocks while the rest transfers
                        nk = 2 if (b, h) == (1, 1) else 1
                        for kk in range(nk):
                            ksl = slice(kk * KC // nk, (kk + 1) * KC // nk)
                            csl = slice(kk * (KC // nk) * CW,
                                        (kk + 1) * (KC // nk) * CW)
                            nc.sync.dma_start(
                                rh_b[h * HD:(h + 1) * HD, csl].rearrange(
                                    "p (kc c) -> p kc c", c=CW),
                                a2a_out[b][h][ksl].rearrange(
                                    "kc p c -> p kc c"))
                    return rh_b

                def proj_batch(b, rh_b, ps4, outs):
                    # batch 1 finishes with two single-mc groups so the
                    # closing bias-add + out-DMA chain is half as long
                    groups = [(0, 2), (2, 2), (4, 2)] + (
                        [(6, 1), (7, 1)] if b == 1 else [(6, 2)])
                    for mc0, w_ in groups:
                        ps = ps4.tile([P, 2 * CW], F32, tag="ps4",
                                      name=f"ps4_{it}_{b}_{mc0}")
                        for half in range(w_):
                            mc = mc0 + half
                            for kc in range(KC):
                                nc.tensor.matmul(
                                    ps[:, half * CW:(half + 1) * CW],
                                    wo_s[kc][:, mc * P:(mc + 1) * P],
                                    rh_b[:, kc * CW:(kc + 1) * CW],
                                    start=(kc == 0), stop=(kc == KC - 1))
                        ot = outs.tile([P, 2 * CW], F32, tag="ot",
                                       name=f"ot_{it}_{b}_{mc0}")
                        for half in range(w_):
                            mc = mc0 + half
                            osl = slice(half * CW, (half + 1) * CW)
                            nc.vector.tensor_scalar_add(ot[:, osl],
                                                        ps[:, osl],
                                                        bo_s[:, mc:mc + 1])
                        nc.sync.dma_start(
                            out[mc0 * P:(mc0 + w_) * P,
                                b * CW:(b + 1) * CW].rearrange(
                                    "(two p) c -> p two c", p=P),
                            ot[:, 0:w_ * CW].rearrange(
                                "p (two c) -> p two c", c=CW))

                with tc.tile_pool(name=f"xt{it}", bufs=32) as xt_pool:
                    # ---- batch-0 QKV + V-transposes (full-width PSUM) ----
                    with (
                        tc.tile_pool(name=f"ps1{it}", bufs=6,
                                     space="PSUM") as ps1,
                        tc.tile_pool(name=f"pst{it}", bufs=2,
                                     space="PSUM") as pst,
                    ):
                        xts0 = load_half(0, xt_pool)
                        # first round: all three projections for nb0/nb1
                        # (matches the x DMA feed rate -- a PE stall resets
                        # the p-state ramp); then [K,V] for nb2/nb3 before
                        # [Q] so the ps1 slots that attention's lg tiles
                        # inherit (banks 0-3) free ~3us before Q's copies
                        rounds = [[(0, 0), (0, 1), (1, 0), (1, 1),
                                   (2, 0), (2, 1)],
                                  [(0, 2), (0, 3)],
                                  [(2, 2), (2, 3)],
                                  [(1, 2), (1, 3)]]
                        for items in rounds:
                            pss = [ps1.tile([P, SB], F32, tag="ps1",
                                            name=f"ps1_{it}_{pj}_{nb}")
                                   for pj, nb in items]
                            for kc in range(KC):
                                for t, (pj, nb) in zip(pss, items):
                                    nc.tensor.matmul(
                                        t[:],
                                        W_OF[pj][:, kc * P:(kc + 1) * P],
                                        xts0[(kc, nb)],
                                        start=(kc == 0), stop=(kc == KC - 1))
                            for t, (pj, nb) in zip(pss, items):
                                bc = BIAS_COL[pj]
                                dst = DST_OF[pj][:, nb * SB:(nb + 1) * SB]
                                # Q nb2/nb3 copies both go to ACT: it idles
                                # in the settle window, and pv's bank WAR
                                # waits on exactly these copies
                                on_act = ((pj + nb) % 2 == 1
                                          or (pj, nb) == (1, 3))
                                if not on_act:
                                    nc.vector.tensor_scalar_add(
                                        dst, t[:], bias3[:, bc:bc + 1])
                                else:
                                    nc.scalar.add(dst, t[:],
                                                  bias3[:, bc:bc + 1])
                        vnat(0, pst, "pst")

                    for kc in range(KC):
                        if bench:
                            nc.vector.memset(wo_s[kc][:], 0.0)
                        else:
                            nc.sync.dma_start(
                                wo_s[kc][:], woT[kc * P:(kc + 1) * P, :])
                    if phases < 2:
                        continue

                    with (
                        tc.tile_pool(name=f"ps3{it}", bufs=1,
                                     space="PSUM") as ps3,
                        tc.tile_pool(name=f"exps{it}", bufs=7) as exps,
                        tc.tile_pool(name=f"norm{it}", bufs=2) as norm,
                    ):
                        # attention b0 (6 banks); the scheduler back-fills
                        # PE's exp-wait slack with the QKV-half1 stream below
                        attention_batch(0, ps3, exps, norm)

                        with tc.tile_pool(name=f"ps1b{it}", bufs=2,
                                          space="PSUM") as ps1b:
                            with tc.tile_wait_until(0.022):
                                xts1 = load_half(1, xt_pool)
                            with tc.tile_wait_until(0.028):
                                for pj, nb in QKV_ORDER:
                                    qkv_group(pj, nb, R // 2, xts1, ps1b,
                                              "ps1b", nc.vector)
                        with tc.tile_pool(name=f"pstb{it}", bufs=2,
                                          space="PSUM") as pstb:
                            with tc.tile_wait_until(0.036):
                                vnat(1, pstb, "pstb")

                        attention_batch(1, ps3, exps, norm)

                        if phases < 3:
                            continue
                        # projection PSUM reuses ps1b's 2 banks (free since
                        # mid-attention) so these matmuls can run inside the
                        # final AllToAll window
                        with (
                            tc.tile_pool(name=f"ps4{it}", bufs=2,
                                         space="PSUM") as ps4,
                            tc.tile_pool(name=f"proj{it}", bufs=1) as proj,
                            tc.tile_pool(name=f"outs{it}", bufs=4) as outs,
                        ):
                            rh0 = gather_rh(0, proj)
                            rh1 = gather_rh(1, proj)
                            with tc.tile_wait_until(0.145):
                                proj_batch(0, rh0, ps4, outs)
                            # dependency-free matmuls bridge the final
                            # AllToAll window so the tensor engine's p-state
                            # ramp stays hot for batch-1's projection
                            with tc.tile_wait_until(0.150):
                                for wm in range(135):
                                    wt = ps4.tile([P, 2 * CW], F32,
                                                  tag="ps4",
                                                  name=f"warm_{it}_{wm}")
                                    nc.tensor.matmul(
                                        wt[:], w_k[:, 0:P],
                                        attnT[:, 0:2 * CW],
                                        start=True, stop=True)
                            with tc.tile_wait_until(0.155):
                                proj_batch(1, rh1, ps4, outs)

    nc.compile()
    return nc


def _get_program(n_iters=1, phases=3, bench=False):
    key = (n_iters, phases, bench)
    if key not in _CACHE:
        _CACHE[key] = _build(n_iters, phases, bench)
    return _CACHE[key]


def _w_swizzle(w, sl):
    # device tile layout [P, KC*P]: tile[p, kc*P + c] = w[sl][c, kc*P + p]
    wT = np.asarray(w, np.float32)[sl, :].T.astype(np.float16)  # [D, DL]
    return np.ascontiguousarray(
        wT.reshape(KC, P, DL).transpose(1, 0, 2).reshape(P, D))


def _in_maps(x, wq, bq, wk, bk, wv, bv, wo, bo):
    x = np.asarray(x, np.float32)
    xT = np.ascontiguousarray(x.reshape(R, D).T.astype(np.float16))
    woT = np.ascontiguousarray(
        np.asarray(wo, np.float32).T.astype(np.float16))
    bo_t = np.ascontiguousarray(
        np.asarray(bo, np.float32).reshape(NCORES, P).T)
    maps = []
    for i in range(NCORES):
        sl = slice(i * DL, (i + 1) * DL)
        maps.append({
            "xT": xT,
            "wqS": _w_swizzle(wq, sl),
            "wkS": _w_swizzle(wk, sl),
            "wvS": _w_swizzle(wv, sl),
            "woT": woT,
            "bqkv": np.ascontiguousarray(np.stack(
                [np.asarray(bq, np.float32)[sl],
                 np.asarray(bk, np.float32)[sl],
                 np.asarray(bv, np.float32)[sl]], axis=1)),
            "bo_t": bo_t,
        })
    return maps


def kernel(x, wq, bq, wk, bk, wv, bv, wo, bo, **_):
    nc = _get_program()
    res = run_bass_kernel_spmd(nc, _in_maps(x, wq, bq, wk, bk, wv, bv, wo, bo),
                               list(range(NCORES)))
    # core j holds, for each batch b, output columns
    # [b*2048 + j*256, b*2048 + (j+1)*256) of out.T
    CW = RSL // 2
    outT = np.empty((D, R), np.float32)
    for j in range(NCORES):
        o = res.results[j]["out"]
        for b in range(B):
            outT[:, b * S + j * CW:(b * S) + (j + 1) * CW] = \
                o[:, b * CW:(b + 1) * CW]
    return np.ascontiguousarray(outT.T).reshape(B, S, D)



# revision 30
# speedup vs baseline: 1.0254x; 1.0254x over previous
"""Multi-head attention (B=2, S=2048, D=1024, H=16) on 8 trn2 NeuronCores.

Tensor-parallel over heads (2 heads per core, column-sliced wq/wk/wv) for the
QKV projections and attention; a per-(batch, head-group) AllToAll then
redistributes the attention output so each core computes the output
projection for its own interleaved 512-row slice of the flattened (B*S)
sequence (Megatron-style TP with a sequence-parallel output projection).

Layout/engine choices (timeline-profiled to 225.5us on the v2 cost model,
from a 237.1us starting point):
  - the host supplies x.T and pre-swizzled w tiles so every DMA row is >=
    1KB contiguous (the HWDGE descriptor engine costs 0.62us per DMA, so
    few/large transfers matter); no activation transposes on device
  - x streams in np-major [128,1024] chunks; QKV runs all three projections
    per nb-pair so PE consumption matches the x DMA feed rate -- any PE
    stall resets the tensor engine's p-state ramp to half clock
  - logits are computed transposed [t, s] so the softmax exp (over t) feeds
    the P@V matmul directly -- no probability-matrix transposes
  - ones-columns appended to V produce the softmax denominators in the same
    PV matmul (PSUM rows 64..127), replicated across partitions for a cheap
    vector normalize
  - matmuls run in float32r (full-rate relaxed fp32); the x/w stream and the
    projection tail (attnT, collective buffers, wo) are float16
  - exp runs on ACT from 2x[128,1024] double-buffered PSUM logit tiles; ACT
    paces attention at ~1.04us/tile vs PE's 0.85us, and the tile scheduler
    back-fills PE's slack with the batch-1 QKV stream (tile_wait_until pins
    keep the compile-time scheduler from ordering that stream ahead of
    attention, whose x arrives later than the scheduler's DMA model thinks)
  - attention's 6 PSUM banks are placed on banks whose phase-A tenants die
    early; the 2 QKV-half1 banks are recycled for the output projection so
    its matmuls are not WAR-blocked behind attention's last PSUM reads
  - the four 0.25MB AllToAlls (15us constant + 40GB/s each, serialized on
    the collective engine) overlap attention; only the last is exposed.
    Batch-0's output projection plus a stream of dependency-free warm-up
    matmuls bridge that window so the p-state ramp is still hot when
    batch-1's projection runs behind the final collective; the final
    normalize reads PSUM directly and ships as a 128KB slice-pair, and
    batch-1's projection ends in single-mc groups to shorten the closing
    bias-add + out-DMA chain
"""

import sys

sys.path.insert(0, "/opt/trn_rl_repo")

import numpy as np

import concourse.mybir as mybir
import concourse.tile as tile
from concourse import bacc
from concourse.bass_utils import run_bass_kernel_spmd
from concourse.masks import make_identity

B, S, D = 2, 2048, 1024
H, HD = 16, 64
NCORES = 8
DL = D // NCORES          # 128 local attn dims (2 heads) per core
R = B * S                 # 4096 flattened rows
RSL = R // NCORES         # 512 output rows per core
P = 128
KC = D // P               # 8 contraction chunks of 128
TC = S // P               # 16 key/t chunks per batch
SB = 512                  # moving-operand (N) tile
NSB = (R // 2) // SB      # 4 row-chunks per half
F32 = mybir.dt.float32
F32R = mybir.dt.float32r
F16 = mybir.dt.float16
I16 = mybir.dt.int16

_CACHE = {}


def _build(n_iters=1, phases=3, bench=False):
    nc = bacc.Bacc("TRN2", target_bir_lowering=False, debug=False,
                   num_devices=NCORES)
    Exp = mybir.ActivationFunctionType.Exp

    LOG2E = 1.4426950408889634
    SCH_A = LOG2E * 1024.0 / 8.0      # fold the 1/8 logit scale
    SCH_B = 15360.0 - 44.0            # f16 exponent bias - rms-optimal c
    # t-chunks whose exp runs on DVE (Schraudolph): b0's window is
    # stretched by the QKV backfill so ACT keeps 14/16 there (fewer
    # approximate exps = better accuracy); b1 is PE-paced so ACT can
    # only carry 10/16
    DVE_TCN_B = {0: (5, 11), 1: (2, 4, 7, 9, 12, 14)}

    kind = "Internal" if bench else "ExternalInput"
    xT = nc.dram_tensor("xT", [D, R], F16, kind=kind)
    # w*S are pre-swizzled on host to the SBUF tile layout [P, KC*P]
    wqS = nc.dram_tensor("wqS", [P, D], F16, kind=kind)
    wkS = nc.dram_tensor("wkS", [P, D], F16, kind=kind)
    wvS = nc.dram_tensor("wvS", [P, D], F16, kind=kind)
    woT = nc.dram_tensor("woT", [D, D], F16, kind=kind)
    bqkv = nc.dram_tensor("bqkv", [DL, 3], F32, kind=kind)
    bo_t = nc.dram_tensor("bo_t", [P, NCORES], F32, kind=kind)
    out = nc.dram_tensor("out", [D, RSL], F32, kind="ExternalOutput")

    with tile.TileContext(nc) as tc:
        with (
            tc.tile_pool(name="const", bufs=1) as const,
            tc.tile_pool(name="persist", bufs=1) as persist,
            tc.tile_pool(name="dram", bufs=1, space="DRAM") as dram,
        ):
            # ---- constants / weights resident in SBUF ----
            w_s = []
            for name, wt in (("wk", wkS), ("wq", wqS), ("wv", wvS)):
                t = const.tile([P, D], F16, tag=f"w_{name}", name=f"w_{name}")
                if bench:
                    nc.vector.memset(t[:], 0.0)
                w_s.append(t)
            w_k, w_q, w_v = w_s

            def load_weights():
                # issued after the first x chunk: the serialized DMA queue
                # then delivers (x kc0, w_k) first so round-0 starts ~2.3us
                # instead of queueing x behind all three weight tensors
                if not bench:
                    for t, wt in zip(w_s, (wkS, wqS, wvS)):
                        nc.sync.dma_start(t[:], wt[:, :])
                    nc.sync.dma_start(bias3[:], bqkv[:])
                    nc.sync.dma_start(bo_s[:], bo_t[:])

            ident = const.tile([P, P], F16, tag="ident")
            make_identity(nc, ident[:])
            bias3 = const.tile([DL, 3], F32, tag="bias3")
            bo_s = const.tile([P, NCORES], F32, tag="bo_s")
            if bench:
                nc.vector.memset(bias3[:], 0.0)
                nc.vector.memset(bo_s[:], 0.0)
            wo_s = [const.tile([P, D], F16, tag=f"wo{kc}", name=f"wo{kc}")
                    for kc in range(KC)]

            # persistent activations
            QT = persist.tile([P, R], F32R, tag="QT")   # [2 heads*64, B*S]
            KT = persist.tile([P, R], F32R, tag="KT")
            VT = persist.tile([P, R], F16, tag="VT")
            # V natural per 128-row t-chunk: [v_h0 |ones| v_h1 |ones]
            vn = persist.tile([P, (R // P) * 256], F16, tag="vn")
            vn3 = vn[:].rearrange("p (g two c) -> p g two c", two=2, c=128)
            nc.gpsimd.memset(vn3[:, :, :, 64:128], 1.0)
            attnT = persist.tile([P, R], F16, tag="attnT")

            # QKV issue order: K first (logits sweep every t-chunk, so K has
            # the earliest deadline), then Q for the first s-half, V; the
            # Q-sh1 groups are deferred into batch-1's attention window as
            # PE filler (b1 has no other backfill for the exp-chain slack)
            QKV_ORDER = ([(0, nb) for nb in range(NSB)]          # K
                         + [(2, nb) for nb in range(NSB)]        # V
                         + [(1, 0), (1, 1)]                      # Q sh0
                         + [(1, 2), (1, 3)])                     # Q sh1
            W_OF = {0: w_k, 1: w_q, 2: w_v}
            DST_OF = {0: KT, 1: QT, 2: VT}
            BIAS_COL = {0: 1, 1: 0, 2: 2}   # bias3 columns are (q, k, v)

            for it in range(n_iters):
                SH = S // 2
                CW = RSL // 2
                a2a_in = [[dram.tile([NCORES, HD, CW], F16,
                                     tag=f"a2a_in{it}_{b}_{h}",
                                     name=f"a2a_in{it}_{b}_{h}")
                           for h in range(2)] for b in range(B)]
                a2a_out = [[dram.tile([NCORES, HD, CW], F16,
                                      tag=f"a2a_out{it}_{b}_{h}",
                                      name=f"a2a_out{it}_{b}_{h}")
                            for h in range(2)] for b in range(B)]

                def load_half(half, xt_pool, after_first=None):
                    # np-major [128,1024] chunks: the first QKV round is
                    # DMA-complete after ~2MB, and DMA count stays low (the
                    # HWDGE descriptor engine costs 0.62us per DMA)
                    hof = half * (R // 2)
                    xts = {}
                    for np_ in range(2):
                        for kc in range(KC):
                            t = xt_pool.tile([P, 2 * SB], F16, tag="xt",
                                             name=f"xt_{it}_{half}_{np_}_{kc}")
                            nc.sync.dma_start(
                                t[:], xT[kc * P:(kc + 1) * P,
                                         hof + np_ * 2 * SB:
                                         hof + (np_ + 1) * 2 * SB])
                            if after_first is not None:
                                after_first()
                                after_first = None
                            for i in range(2):
                                xts[(kc, np_ * 2 + i)] = t[:, i * SB:
                                                           (i + 1) * SB]
                    return xts

                def qkv_group(pj, nb, hof, xts, pool, tag, eng):
                    t = pool.tile([P, SB], F32, tag=tag,
                                  name=f"{tag}_{it}_{hof}_{pj}_{nb}")
                    for kc in range(KC):
                        nc.tensor.matmul(
                            t[:], W_OF[pj][:, kc * P:(kc + 1) * P],
                            xts[(kc, nb)],
                            start=(kc == 0), stop=(kc == KC - 1))
                    bc = BIAS_COL[pj]
                    eng.tensor_scalar_add(
                        DST_OF[pj][:, hof + nb * SB:hof + (nb + 1) * SB],
                        t[:], bias3[:, bc:bc + 1])

                def vnat4(g0, pool, tag, bufs=1):
                    # V natural for 4 t-chunks: 4 transposes into one
                    # [128,512] f16 PSUM bank + a single strided combined
                    # copy (f16 both sides -> DVE 2x mode, ~0.4us per 4)
                    pt4 = pool.tile([P, 4 * P], F16, tag=tag, bufs=bufs,
                                    name=f"pt4_{it}_{g0}")
                    for q in range(4):
                        nc.tensor.transpose(
                            pt4[:, q * P:(q + 1) * P],
                            VT[:, (g0 + q) * P:(g0 + q + 1) * P], ident[:])
                    src = pt4[:].rearrange("p (q db di) -> p q db di",
                                           q=4, db=2)
                    dst = vn3[:, g0:g0 + 4, :, 0:64]
                    nc.vector.tensor_copy(dst, src)

                def attention_batch(b, pvp, lgp, lgbufs, exps, norm):
                    base = b * S
                    DVE_TCN = DVE_TCN_B[b]
                    lg_pre = []
                    if b == 0:
                        # fix the lg tag's two ring slots on banks 0-3
                        # (ps1's early-freed slots) before pv claims them
                        lg_pre = [lgp.tile([P, SH], F32, tag="lg", bufs=lgbufs,
                                           name=f"lg_pre_{it}_{k}")
                                  for k in range(2)]
                    for h in range(2):
                        hr = slice(h * HD, (h + 1) * HD)
                        for sh in range(2):
                            sof = base + sh * SH
                            pv = pvp.tile([P, SH], F32, tag="pv", bufs=1,
                                          name=f"pv_{it}_{b}_{h}_{sh}")

                            def emit_logits(tcn):
                                lg = (lg_pre.pop(0) if lg_pre else
                                      lgp.tile([P, SH], F32, tag="lg",
                                               bufs=lgbufs,
                                               name=f"lg_{it}_{b}_{h}_{sh}_{tcn}"))
                                for sb in range(2):
                                    nc.tensor.matmul(
                                        lg[:, sb * SB:(sb + 1) * SB],
                                        KT[hr, base + tcn * P:
                                           base + (tcn + 1) * P],
                                        QT[hr, sof + sb * SB:
                                           sof + (sb + 1) * SB],
                                        start=True, stop=True)
                                return lg

                            def emit_exp_pv(tcn, lg):
                                if tcn in DVE_TCN:
                                    # Schraudolph exp on DVE: i16 bit-trick,
                                    # bitcast to f16 for the PV matmul
                                    exd = exps.tile([P, SH], I16, tag="exd",
                                                    bufs=3,
                                                    name=f"exd_{it}_{b}_{h}_{sh}_{tcn}")
                                    nc.vector.tensor_scalar(
                                        exd[:], lg[:], SCH_A, SCH_B,
                                        mybir.AluOpType.mult,
                                        mybir.AluOpType.add)
                                    ex = exd[:].bitcast(F16)
                                else:
                                    ext = exps.tile([P, SH], F16, tag="ex",
                                                    name=f"ex_{it}_{b}_{h}_{sh}_{tcn}")
                                    nc.scalar.activation(ext[:], lg[:], Exp,
                                                         scale=1.0 / 8.0)
                                    ex = ext[:]
                                o = (b * TC + tcn) * 256 + h * 128
                                for sb in range(2):
                                    nc.tensor.matmul(
                                        pv[:, sb * SB:(sb + 1) * SB],
                                        vn[:, o:o + 128],
                                        ex[:, sb * SB:(sb + 1) * SB],
                                        start=(tcn == 0), stop=(tcn == TC - 1))

                            # software-pipelined emission: logits(k+1) goes
                            # into the (in-order) PE stream before PV(k), so
                            # PE never sits behind PV's wait on exp(k)
                            prev = (0, emit_logits(0))
                            for tcn in range(1, TC):
                                lg = emit_logits(tcn)
                                emit_exp_pv(prev[0], prev[1])
                                prev = (tcn, lg)
                            emit_exp_pv(prev[0], prev[1])
                            # normalize straight out of PSUM: the next
                            # chunk's first PV matmul (its WAW on this bank)
                            # only comes ~2us later, after logits+exp
                            rc = norm.tile([HD, SH], F32, tag="rc",
                                           name=f"rc_{it}_{b}_{h}_{sh}")
                            nc.vector.reciprocal(rc[:], pv[64:128, :])
                            nc.vector.tensor_mul(
                                attnT[h * HD:(h + 1) * HD, sof:sof + SH],
                                pv[0:64, :], rc[:])
                            # ship the finished half-row-block right away:
                            # the final a2a then waits only on a 128KB DMA
                            if phases >= 3:
                                nc.sync.dma_start(
                                    a2a_in[b][h][4 * sh:4 * sh + 4]
                                    .rearrange("j p c -> p j c"),
                                    attnT[h * HD:(h + 1) * HD,
                                          sof:sof + SH].rearrange(
                                              "p (j c) -> p j c", c=CW))
                        if phases >= 3:
                            nc.gpsimd.collective_compute(
                                "AllToAll", mybir.AluOpType.bypass,
                                replica_groups=[list(range(NCORES))],
                                ins=[a2a_in[b][h].opt()],
                                outs=[a2a_out[b][h].opt()])

                def gather_rh(b, proj):
                    rh_b = proj.tile([P, KC * CW], F16, tag=f"rh{it}_{b}",
                                     name=f"rh{it}_{b}")
                    for h in range(2):
                        # batch-1 h1 lands last: gather it in two halves so
                        # the projection can start on the first four
                        # kc-blocks while the rest transfers
                        nk = 2 if (b, h) == (1, 1) else 1
                        for kk in range(nk):
                            ksl = slice(kk * KC // nk, (kk + 1) * KC // nk)
                            csl = slice(kk * (KC // nk) * CW,
                                        (kk + 1) * (KC // nk) * CW)
                            nc.sync.dma_start(
                                rh_b[h * HD:(h + 1) * HD, csl].rearrange(
                                    "p (kc c) -> p kc c", c=CW),
                                a2a_out[b][h][ksl].rearrange(
                                    "kc p c -> p kc c"))
                    return rh_b

                def proj_batch(b, rh_b, ps4, outs):
                    # batch 1 finishes with two single-mc groups so the
                    # closing bias-add + out-DMA chain is half as long
                    groups = [(0, 2), (2, 2), (4, 2)] + (
                        [(6, 1), (7, 1)] if b == 1 else [(6, 2)])
                    for mc0, w_ in groups:
                        ps = ps4.tile([P, 2 * CW], F32, tag="ps4",
                                      name=f"ps4_{it}_{b}_{mc0}")
                        for half in range(w_):
                            mc = mc0 + half
                            for kc in range(KC):
                                nc.tensor.matmul(
                                    ps[:, half * CW:(half + 1) * CW],
                                    wo_s[kc][:, mc * P:(mc + 1) * P],
                                    rh_b[:, kc * CW:(kc + 1) * CW],
                                    start=(kc == 0), stop=(kc == KC - 1))
                        ot = outs.tile([P, 2 * CW], F32, tag="ot",
                                       name=f"ot_{it}_{b}_{mc0}")
                        for half in range(w_):
                            mc = mc0 + half
                            osl = slice(half * CW, (half + 1) * CW)
                            nc.vector.tensor_scalar_add(ot[:, osl],
                                                        ps[:, osl],
                                                        bo_s[:, mc:mc + 1])
                        nc.sync.dma_start(
                            out[mc0 * P:(mc0 + w_) * P,
                                b * CW:(b + 1) * CW].rearrange(
                                    "(two p) c -> p two c", p=P),
                            ot[:, 0:w_ * CW].rearrange(
                                "p (two c) -> p two c", c=CW))

                with tc.tile_pool(name=f"xt{it}", bufs=32) as xt_pool:
                    # ---- batch-0 QKV + V-transposes (full-width PSUM) ----
                    with (
                        tc.tile_pool(name=f"ps1{it}", bufs=6,
                                     space="PSUM") as ps1,
                    ):
                        xts0 = load_half(
                            0, xt_pool,
                            after_first=load_weights if it == 0 else None)
                        # first round: all three projections for nb0/nb1
                        # (matches the x DMA feed rate -- a PE stall resets
                        # the p-state ramp); then [K,V] for nb2/nb3 before
                        # [Q] so the ps1 slots that attention's lg tiles
                        # inherit (banks 0-3) free ~3us before Q's copies
                        rounds = [[(0, 0), (0, 1), (1, 0), (1, 1),
                                   (2, 0), (2, 1)],
                                  [(0, 2), (0, 3)],
                                  [(2, 2), (2, 3)],
                                  [(1, 2), (1, 3)]]
                        for ri, items in enumerate(rounds):
                            pss = [ps1.tile([P, SB], F32, tag="ps1",
                                            name=f"ps1_{it}_{pj}_{nb}")
                                   for pj, nb in items]
                            for kc in range(KC):
                                for t, (pj, nb) in zip(pss, items):
                                    nc.tensor.matmul(
                                        t[:],
                                        W_OF[pj][:, kc * P:(kc + 1) * P],
                                        xts0[(kc, nb)],
                                        start=(kc == 0), stop=(kc == KC - 1))
                            for t, (pj, nb) in zip(pss, items):
                                bc = BIAS_COL[pj]
                                dst = DST_OF[pj][:, nb * SB:(nb + 1) * SB]
                                # Q nb2/nb3 copies both go to ACT: it idles
                                # in the settle window, and pv's bank WAR
                                # waits on exactly these copies
                                on_act = ((pj + nb) % 2 == 1
                                          or (pj, nb) == (1, 3))
                                if not on_act:
                                    nc.vector.tensor_scalar_add(
                                        dst, t[:], bias3[:, bc:bc + 1])
                                else:
                                    nc.scalar.add(dst, t[:],
                                                  bias3[:, bc:bc + 1])
                            # V-natural transposes ride the round stream
                            # (V01 ready after round 0, V23 after round 2)
                            if ri == 1:
                                vnat4(0, ps1, "pt4a", bufs=2)
                                vnat4(4, ps1, "pt4a", bufs=2)
                            elif ri == 2:
                                vnat4(8, ps1, "pt4a", bufs=2)
                                vnat4(12, ps1, "pt4a", bufs=2)

                    for kc in range(KC):
                        if bench:
                            nc.vector.memset(wo_s[kc][:], 0.0)
                        else:
                            nc.sync.dma_start(
                                wo_s[kc][:], woT[kc * P:(kc + 1) * P, :])
                    if phases < 2:
                        continue

                    with (
                        tc.tile_pool(name=f"pvp{it}", bufs=1,
                                     space="PSUM") as pvp,
                        tc.tile_pool(name=f"exps{it}", bufs=7) as exps,
                        tc.tile_pool(name=f"norm{it}", bufs=2) as norm,
                    ):
                        with tc.tile_pool(name=f"lga{it}", bufs=1,
                                          space="PSUM") as lga:
                            # attention b0 (2+4 banks); the scheduler
                            # back-fills PE's exp-wait slack with the
                            # QKV-half1 stream below
                            attention_batch(0, pvp, lga, 2, exps, norm)

                            with tc.tile_pool(name=f"ps1b{it}", bufs=1,
                                              space="PSUM") as ps1b:
                                with tc.tile_wait_until(0.022):
                                    xts1 = load_half(1, xt_pool)
                                with tc.tile_wait_until(0.028):
                                    for pj, nb in QKV_ORDER:
                                        # V copies go to ACT: DVE's queue
                                        # (exps+norms) otherwise delays the
                                        # vn chain b1's first PVs need
                                        eng = (nc.scalar if pj == 2
                                               else nc.vector)
                                        qkv_group(pj, nb, R // 2, xts1, ps1b,
                                                  "ps1b", eng)
                                        if pj == 2:
                                            # V-natural transposes for this
                                            # nb ride the backfill stream
                                            vnat4(16 + nb * 4, ps1b, "pt4b")

                        with tc.tile_pool(name=f"lgb{it}", bufs=1,
                                          space="PSUM") as lgb:
                            # b1 has no QKV backfill: a 3-deep lg ring (6
                            # banks, using ps1b's freed pair) lets PE run
                            # logits ~2 chunks ahead of the exp chain
                            attention_batch(1, pvp, lgb, 3, exps, norm)

                        if phases < 3:
                            continue
                        # projection PSUM reuses ps1b's 2 banks (free since
                        # mid-attention) so these matmuls can run inside the
                        # final AllToAll window
                        with (
                            tc.tile_pool(name=f"ps4{it}", bufs=2,
                                         space="PSUM") as ps4,
                            tc.tile_pool(name=f"proj{it}", bufs=1) as proj,
                            tc.tile_pool(name=f"outs{it}", bufs=4) as outs,
                        ):
                            rh0 = gather_rh(0, proj)
                            rh1 = gather_rh(1, proj)
                            with tc.tile_wait_until(0.145):
                                proj_batch(0, rh0, ps4, outs)
                            # dependency-free matmuls bridge the final
                            # AllToAll window so the tensor engine's p-state
                            # ramp stays hot for batch-1's projection
                            with tc.tile_wait_until(0.150):
                                for wm in range(100):
                                    wt = ps4.tile([P, 2 * CW], F32,
                                                  tag="ps4",
                                                  name=f"warm_{it}_{wm}")
                                    nc.tensor.matmul(
                                        wt[:], w_k[:, 0:P],
                                        attnT[:, 0:2 * CW],
                                        start=True, stop=True)
                            with tc.tile_wait_until(0.155):
                                proj_batch(1, rh1, ps4, outs)

    nc.compile()
    return nc


def _get_program(n_iters=1, phases=3, bench=False):
    key = (n_iters, phases, bench)
    if key not in _CACHE:
        _CACHE[key] = _build(n_iters, phases, bench)
    return _CACHE[key]


def _w_swizzle(w, sl):
    # device tile layout [P, KC*P]: tile[p, kc*P + c] = w[sl][c, kc*P + p]
    wT = np.asarray(w, np.float32)[sl, :].T.astype(np.float16)  # [D, DL]
    return np.ascontiguousarray(
        wT.reshape(KC, P, DL).transpose(1, 0, 2).reshape(P, D))


def _in_maps(x, wq, bq, wk, bk, wv, bv, wo, bo):
    x = np.asarray(x, np.float32)
    xT = np.ascontiguousarray(x.reshape(R, D).T.astype(np.float16))
    woT = np.ascontiguousarray(
        np.asarray(wo, np.float32).T.astype(np.float16))
    bo_t = np.ascontiguousarray(
        np.asarray(bo, np.float32).reshape(NCORES, P).T)
    maps = []
    for i in range(NCORES):
        sl = slice(i * DL, (i + 1) * DL)
        maps.append({
            "xT": xT,
            "wqS": _w_swizzle(wq, sl),
            "wkS": _w_swizzle(wk, sl),
            "wvS": _w_swizzle(wv, sl),
            "woT": woT,
            "bqkv": np.ascontiguousarray(np.stack(
                [np.asarray(bq, np.float32)[sl],
                 np.asarray(bk, np.float32)[sl],
                 np.asarray(bv, np.float32)[sl]], axis=1)),
            "bo_t": bo_t,
        })
    return maps


def kernel(x, wq, bq, wk, bk, wv, bv, wo, bo, **_):
    nc = _get_program()
    res = run_bass_kernel_spmd(nc, _in_maps(x, wq, bq, wk, bk, wv, bv, wo, bo),
                               list(range(NCORES)))
    # core j holds, for each batch b, output columns
    # [b*2048 + j*256, b*2048 + (j+1)*256) of out.T
    CW = RSL // 2
    outT = np.empty((D, R), np.float32)
    for j in range(NCORES):
        o = res.results[j]["out"]
        for b in range(B):
            outT[:, b * S + j * CW:(b * S) + (j + 1) * CW] = \
                o[:, b * CW:(b + 1) * CW]
    return np.ascontiguousarray(outT.T).reshape(B, S, D)



# revision 71
# speedup vs baseline: 1.0660x; 1.0396x over previous
"""Multi-head attention (B=2, S=2048, D=1024, H=16) on 8 trn2 NeuronCores.

Tensor-parallel over heads (2 heads per core, column-sliced wq/wk/wv) for the
QKV projections and attention; a per-(batch, head-group) AllToAll then
redistributes the attention output so each core computes the output
projection for its own interleaved 512-row slice of the flattened (B*S)
sequence (Megatron-style TP with a sequence-parallel output projection).

Layout/engine choices (timeline-profiled to 225.5us on the v2 cost model,
from a 237.1us starting point):
  - the host supplies x.T and pre-swizzled w tiles so every DMA row is >=
    1KB contiguous (the HWDGE descriptor engine costs 0.62us per DMA, so
    few/large transfers matter); no activation transposes on device
  - x streams in np-major [128,1024] chunks; QKV runs all three projections
    per nb-pair so PE consumption matches the x DMA feed rate -- any PE
    stall resets the tensor engine's p-state ramp to half clock
  - logits are computed transposed [t, s] so the softmax exp (over t) feeds
    the P@V matmul directly -- no probability-matrix transposes
  - ones-columns appended to V produce the softmax denominators in the same
    PV matmul (PSUM rows 64..127), replicated across partitions for a cheap
    vector normalize
  - matmuls run in float32r (full-rate relaxed fp32); the x/w stream and the
    projection tail (attnT, collective buffers, wo) are float16
  - exp runs on ACT from 2x[128,1024] double-buffered PSUM logit tiles; ACT
    paces attention at ~1.04us/tile vs PE's 0.85us, and the tile scheduler
    back-fills PE's slack with the batch-1 QKV stream (tile_wait_until pins
    keep the compile-time scheduler from ordering that stream ahead of
    attention, whose x arrives later than the scheduler's DMA model thinks)
  - attention's 6 PSUM banks are placed on banks whose phase-A tenants die
    early; the 2 QKV-half1 banks are recycled for the output projection so
    its matmuls are not WAR-blocked behind attention's last PSUM reads
  - the four 0.25MB AllToAlls (15us constant + 40GB/s each, serialized on
    the collective engine) overlap attention; only the last is exposed.
    Batch-0's output projection plus a stream of dependency-free warm-up
    matmuls bridge that window so the p-state ramp is still hot when
    batch-1's projection runs behind the final collective; the final
    normalize reads PSUM directly and ships as a 128KB slice-pair, and
    batch-1's projection ends in single-mc groups to shorten the closing
    bias-add + out-DMA chain
"""

import sys

sys.path.insert(0, "/opt/trn_rl_repo")

import numpy as np

import concourse.mybir as mybir
import concourse.tile as tile
from concourse import bacc
from concourse.bass_utils import run_bass_kernel_spmd
from concourse.masks import make_identity

B, S, D = 2, 2048, 1024
H, HD = 16, 64
NCORES = 8
DL = D // NCORES          # 128 local attn dims (2 heads) per core
R = B * S                 # 4096 flattened rows
RSL = R // NCORES         # 512 output rows per core
P = 128
KC = D // P               # 8 contraction chunks of 128
TC = S // P               # 16 key/t chunks per batch
SB = 512                  # moving-operand (N) tile
NSB = (R // 2) // SB      # 4 row-chunks per half
CW0 = RSL // 2            # per-batch output column count
F32 = mybir.dt.float32
F32R = mybir.dt.float32r
F16 = mybir.dt.float16
I16 = mybir.dt.int16

_CACHE = {}


def _build(n_iters=1, phases=3, bench=False):
    nc = bacc.Bacc("TRN2", target_bir_lowering=False, debug=False,
                   num_devices=NCORES)
    Exp = mybir.ActivationFunctionType.Exp

    LOG2E = 1.4426950408889634
    SCH_A = LOG2E * 1024.0 / 8.0      # fold the 1/8 logit scale
    SCH_B = 15360.0 - 44.0            # f16 exponent bias - rms-optimal c
    # t-chunks whose exp runs on DVE (Schraudolph): b0's window is
    # stretched by the QKV backfill so ACT keeps 14/16 there (fewer
    # approximate exps = better accuracy); b1 is PE-paced so ACT can
    # only carry 10/16
    DVE_TCN_B = {0: (5, 11), 1: (2, 4, 7, 9, 12, 14)}

    kind = "Internal" if bench else "ExternalInput"
    xT = nc.dram_tensor("xT", [D, R], F16, kind=kind)
    # w*S are pre-swizzled on host to the SBUF tile layout [P, KC*P]
    wqS = nc.dram_tensor("wqS", [P, D], F16, kind=kind)
    wkS = nc.dram_tensor("wkS", [P, D], F16, kind=kind)
    wvS = nc.dram_tensor("wvS", [P, D], F16, kind=kind)
    woT = nc.dram_tensor("woT", [D, D], F16, kind=kind)
    bqkv = nc.dram_tensor("bqkv", [DL, 3], F32, kind=kind)
    bo_t = nc.dram_tensor("bo_t", [P, NCORES], F32, kind=kind)
    out = nc.dram_tensor("out", [D, RSL], F16, kind="ExternalOutput")

    with tile.TileContext(nc) as tc:
        with (
            tc.tile_pool(name="const", bufs=1) as const,
            tc.tile_pool(name="persist", bufs=1) as persist,
            tc.tile_pool(name="dram", bufs=1, space="DRAM") as dram,
        ):
            # ---- constants / weights resident in SBUF ----
            w_s = []
            for name, wt in (("wk", wkS), ("wq", wqS), ("wv", wvS)):
                t = const.tile([P, D], F16, tag=f"w_{name}", name=f"w_{name}")
                if bench:
                    nc.vector.memset(t[:], 0.0)
                w_s.append(t)
            w_k, w_q, w_v = w_s

            def load_weights():
                # issued after the first x chunk: the serialized DMA queue
                # then delivers (x kc0, w_k) first so round-0 starts ~2.3us
                # instead of queueing x behind all three weight tensors
                if not bench:
                    for t, wt in zip(w_s, (wkS, wqS, wvS)):
                        nc.sync.dma_start(t[:], wt[:, :])
                    nc.sync.dma_start(bias3[:], bqkv[:])
                    nc.sync.dma_start(bo_s[:], bo_t[:])

            ident = const.tile([P, P], F16, tag="ident")
            make_identity(nc, ident[:])
            bias3 = const.tile([DL, 3], F32, tag="bias3")
            bo_s = const.tile([P, NCORES], F32, tag="bo_s")
            if bench:
                nc.vector.memset(bias3[:], 0.0)
                nc.vector.memset(bo_s[:], 0.0)
            wo_s = [const.tile([P, D], F16, tag=f"wo{kc}", name=f"wo{kc}")
                    for kc in range(KC)]

            # persistent activations
            QT = persist.tile([P, R], F32R, tag="QT")   # [2 heads*64, B*S]
            KT = persist.tile([P, R], F32R, tag="KT")
            VT = persist.tile([P, R], F16, tag="VT")
            # V natural per 128-row t-chunk: [v_h0 |ones| v_h1 |ones]
            vn = persist.tile([P, (R // P) * 256], F16, tag="vn")
            vn3 = vn[:].rearrange("p (g two c) -> p g two c", two=2, c=128)
            nc.gpsimd.memset(vn3[:, :, :, 64:128], 1.0)
            attnT = persist.tile([P, R], F16, tag="attnT")

            # QKV issue order: K first (logits sweep every t-chunk, so K has
            # the earliest deadline), then Q for the first s-half, V; the
            # Q-sh1 groups are deferred into batch-1's attention window as
            # PE filler (b1 has no other backfill for the exp-chain slack)
            QKV_ORDER = ([(0, nb) for nb in range(NSB)]          # K
                         + [(2, nb) for nb in range(NSB)]        # V
                         + [(1, 0), (1, 1)]                      # Q sh0
                         + [(1, 2), (1, 3)])                     # Q sh1
            W_OF = {0: w_k, 1: w_q, 2: w_v}
            DST_OF = {0: KT, 1: QT, 2: VT}
            BIAS_COL = {0: 1, 1: 0, 2: 2}   # bias3 columns are (q, k, v)

            for it in range(n_iters):
                SH = S // 2
                CW = RSL // 2
                a2a_in = [[dram.tile([NCORES, HD, CW], F16,
                                     tag=f"a2a_in{it}_{b}_{h}",
                                     name=f"a2a_in{it}_{b}_{h}")
                           for h in range(2)] for b in range(B)]
                a2a_out = [[dram.tile([NCORES, HD, CW], F16,
                                      tag=f"a2a_out{it}_{b}_{h}",
                                      name=f"a2a_out{it}_{b}_{h}")
                            for h in range(2)] for b in range(B)]

                def load_half(half, xt_pool, after_first=None):
                    # np-major [128,1024] chunks: the first QKV round is
                    # DMA-complete after ~2MB, and DMA count stays low (the
                    # HWDGE descriptor engine costs 0.62us per DMA)
                    hof = half * (R // 2)
                    xts = {}
                    for np_ in range(2):
                        for kc in range(KC):
                            t = xt_pool.tile([P, 2 * SB], F16, tag="xt",
                                             name=f"xt_{it}_{half}_{np_}_{kc}")
                            nc.sync.dma_start(
                                t[:], xT[kc * P:(kc + 1) * P,
                                         hof + np_ * 2 * SB:
                                         hof + (np_ + 1) * 2 * SB])
                            if after_first is not None:
                                after_first()
                                after_first = None
                            for i in range(2):
                                xts[(kc, np_ * 2 + i)] = t[:, i * SB:
                                                           (i + 1) * SB]
                    return xts

                def qkv_group(pj, nb, hof, xts, pool, tag, eng):
                    t = pool.tile([P, SB], F32, tag=tag,
                                  name=f"{tag}_{it}_{hof}_{pj}_{nb}")
                    for kc in range(KC):
                        nc.tensor.matmul(
                            t[:], W_OF[pj][:, kc * P:(kc + 1) * P],
                            xts[(kc, nb)],
                            start=(kc == 0), stop=(kc == KC - 1))
                    bc = BIAS_COL[pj]
                    dst = DST_OF[pj][:, hof + nb * SB:hof + (nb + 1) * SB]
                    if eng is nc.scalar:
                        eng.add(dst, t[:], bias3[:, bc:bc + 1])
                    else:
                        eng.tensor_scalar_add(dst, t[:], bias3[:, bc:bc + 1])

                def vnat4(g0, pool, tag, bufs=1):
                    # V natural for 4 t-chunks: 4 transposes into one
                    # [128,512] f16 PSUM bank + a single strided combined
                    # copy (f16 both sides -> DVE 2x mode, ~0.4us per 4)
                    pt4 = pool.tile([P, 4 * P], F16, tag=tag, bufs=bufs,
                                    name=f"pt4_{it}_{g0}")
                    for q in range(4):
                        nc.tensor.transpose(
                            pt4[:, q * P:(q + 1) * P],
                            VT[:, (g0 + q) * P:(g0 + q + 1) * P], ident[:])
                    src = pt4[:].rearrange("p (q db di) -> p q db di",
                                           q=4, db=2)
                    dst = vn3[:, g0:g0 + 4, :, 0:64]
                    nc.vector.tensor_copy(dst, src)

                def attention_batch(b, pvp, pvbufs, lgp, lgbufs, exps, norm,
                                    filler=None):
                    # One flat software-pipelined stream over the batch's 64
                    # (h, sh, tcn) chunks: logits run `depth` chunks ahead of
                    # exp+PV (depth = lg ring - 1), flowing ACROSS (h, sh)
                    # boundaries so the exp engines never cold-start. pv is
                    # two single-bank tiles (one per sb) so the next group's
                    # first PV only WAR-waits on half the normalize.
                    base = b * S
                    DVE_TCN = DVE_TCN_B[b]
                    depth = lgbufs - 1
                    lg_pre = []
                    if b == 0:
                        # fix the lg tag's two ring slots on banks 0-3
                        # (ps1's early-freed slots) before pv claims them
                        lg_pre = [lgp.tile([P, SH], F32, tag="lg", bufs=lgbufs,
                                           name=f"lg_pre_{it}_{k}")
                                  for k in range(2)]

                    pvt = {}

                    def emit_logits(h, sh, tcn):
                        hr = slice(h * HD, (h + 1) * HD)
                        sof = base + sh * SH
                        lg = (lg_pre.pop(0) if lg_pre else
                              lgp.tile([P, SH], F32, tag="lg", bufs=lgbufs,
                                       name=f"lg_{it}_{b}_{h}_{sh}_{tcn}"))
                        for sb in range(2):
                            nc.tensor.matmul(
                                lg[:, sb * SB:(sb + 1) * SB],
                                KT[hr, base + tcn * P:base + (tcn + 1) * P],
                                QT[hr, sof + sb * SB:sof + (sb + 1) * SB],
                                start=True, stop=True)
                        return lg

                    def emit_exp_pv(h, sh, tcn, lg):
                        if tcn == 0:
                            pvt[(h, sh)] = [
                                pvp.tile([P, SB], F32, tag=f"pv{sb}",
                                         bufs=pvbufs,
                                         name=f"pv{sb}_{it}_{b}_{h}_{sh}")
                                for sb in range(2)]
                        if tcn in DVE_TCN:
                            # Schraudolph exp on DVE: i16 bit-trick,
                            # bitcast to f16 for the PV matmul
                            exd = exps.tile([P, SH], I16, tag="exd", bufs=3,
                                            name=f"exd_{it}_{b}_{h}_{sh}_{tcn}")
                            nc.vector.tensor_scalar(
                                exd[:], lg[:], SCH_A, SCH_B,
                                mybir.AluOpType.mult, mybir.AluOpType.add)
                            ex = exd[:].bitcast(F16)
                        else:
                            ext = exps.tile([P, SH], F16, tag="ex",
                                            name=f"ex_{it}_{b}_{h}_{sh}_{tcn}")
                            nc.scalar.activation(ext[:], lg[:], Exp,
                                                 scale=1.0 / 8.0)
                            ex = ext[:]
                        o = (b * TC + tcn) * 256 + h * 128
                        for sb in range(2):
                            nc.tensor.matmul(
                                pvt[(h, sh)][sb][:],
                                vn[:, o:o + 128],
                                ex[:, sb * SB:(sb + 1) * SB],
                                start=(tcn == 0), stop=(tcn == TC - 1))

                    def emit_norm_ship(h, sh):
                        sof = base + sh * SH
                        for sb in range(2):
                            pv = pvt[(h, sh)][sb]
                            rc = norm.tile([HD, SB], F32, tag=f"rc{sb}",
                                           name=f"rc{sb}_{it}_{b}_{h}_{sh}")
                            nc.vector.reciprocal(rc[:], pv[64:128, :])
                            nc.vector.tensor_mul(
                                attnT[h * HD:(h + 1) * HD,
                                      sof + sb * SB:sof + (sb + 1) * SB],
                                pv[0:64, :], rc[:])
                        # ship the finished half-row-block right away: the
                        # final a2a then waits only on a 128KB DMA
                        if phases >= 3:
                            nc.sync.dma_start(
                                a2a_in[b][h][4 * sh:4 * sh + 4]
                                .rearrange("j p c -> p j c"),
                                attnT[h * HD:(h + 1) * HD,
                                      sof:sof + SH].rearrange(
                                          "p (j c) -> p j c", c=CW))
                        if sh == 1 and phases >= 3:
                            nc.gpsimd.collective_compute(
                                "AllToAll", mybir.AluOpType.bypass,
                                replica_groups=[list(range(NCORES))],
                                ins=[a2a_in[b][h].opt()],
                                outs=[a2a_out[b][h].opt()])

                    chunks = [(h, sh, tcn) for h in range(2)
                              for sh in range(2) for tcn in range(TC)]
                    lgs = {}
                    for i in range(len(chunks) + depth):
                        if i < len(chunks):
                            lgs[i] = emit_logits(*chunks[i])
                        j = i - depth
                        if j >= 0:
                            h, sh, tcn = chunks[j]
                            emit_exp_pv(h, sh, tcn, lgs.pop(j))
                            if tcn == TC - 1:
                                emit_norm_ship(h, sh)
                            if filler is not None:
                                filler(j)

                def gather_rh(b, proj):
                    rh_b = proj.tile([P, KC * CW], F16, tag=f"rh{it}_{b}",
                                     name=f"rh{it}_{b}")
                    for h in range(2):
                        # batch-1 h1 lands last: gather it in two halves so
                        # the projection can start on the first four
                        # kc-blocks while the rest transfers
                        nk = 2 if (b, h) == (1, 1) else 1
                        for kk in range(nk):
                            ksl = slice(kk * KC // nk, (kk + 1) * KC // nk)
                            csl = slice(kk * (KC // nk) * CW,
                                        (kk + 1) * (KC // nk) * CW)
                            nc.sync.dma_start(
                                rh_b[h * HD:(h + 1) * HD, csl].rearrange(
                                    "p (kc c) -> p kc c", c=CW),
                                a2a_out[b][h][ksl].rearrange(
                                    "kc p c -> p kc c"))
                    return rh_b

                def proj_batch(b, rh_b, ps4, outs):
                    # batch 1 finishes with two single-mc groups so the
                    # closing bias + out-DMA chain is half as long. The bias
                    # is a K=1 matmul (bo-row x ones) closing the PSUM
                    # accumulation, and the out-DMA reads PSUM directly --
                    # no SBUF bounce on the tail critical path.
                    groups = [(0, 2), (2, 2), (4, 2)] + (
                        [(6, 1), (7, 1)] if b == 1 else [(6, 2)])
                    for mc0, w_ in groups:
                        ps = ps4.tile([P, 2 * CW], F32, tag="ps4",
                                      name=f"ps4_{it}_{b}_{mc0}")
                        for half in range(w_):
                            mc = mc0 + half
                            for kc in range(KC):
                                nc.tensor.matmul(
                                    ps[:, half * CW:(half + 1) * CW],
                                    wo_s[kc][:, mc * P:(mc + 1) * P],
                                    rh_b[:, kc * CW:(kc + 1) * CW],
                                    start=(kc == 0), stop=(kc == KC - 1))
                        ot = outs.tile([P, 2 * CW], F16, tag="ot",
                                       name=f"ot_{it}_{b}_{mc0}")
                        for half in range(w_):
                            mc = mc0 + half
                            osl = slice(half * CW, (half + 1) * CW)
                            nc.vector.tensor_scalar_add(ot[:, osl],
                                                        ps[:, osl],
                                                        bo_s[:, mc:mc + 1])
                        nc.sync.dma_start(
                            out[mc0 * P:(mc0 + w_) * P,
                                b * CW:(b + 1) * CW].rearrange(
                                    "(two p) c -> p two c", p=P),
                            ot[:, 0:w_ * CW].rearrange(
                                "p (two c) -> p two c", c=CW))

                with tc.tile_pool(name=f"xt{it}", bufs=32) as xt_pool:
                    # ---- batch-0 QKV + V-transposes (full-width PSUM) ----
                    # pt4a gets its own pool so ps1's close (which gates
                    # the attention pools' banks) isn't held by the last
                    # transpose copies
                    with tc.tile_pool(name=f"pt4a{it}", bufs=1,
                                      space="PSUM") as pt4a_pool:
                     with (
                        tc.tile_pool(name=f"ps1{it}", bufs=6,
                                     space="PSUM") as ps1,
                     ):
                        xts0 = load_half(
                            0, xt_pool,
                            after_first=load_weights if it == 0 else None)
                        # first round: all three projections for nb0/nb1
                        # (matches the x DMA feed rate -- a PE stall resets
                        # the p-state ramp); then [K,V] for nb2/nb3 before
                        # [Q] so the ps1 slots that attention's lg tiles
                        # inherit (banks 0-3) free ~3us before Q's copies
                        rounds = [[(0, 0), (0, 1), (1, 0), (1, 1),
                                   (2, 0), (2, 1)],
                                  [(0, 2), (0, 3)],
                                  [(2, 2), (2, 3)],
                                  [(1, 2), (1, 3)]]
                        for ri, items in enumerate(rounds):
                            pss = [ps1.tile([P, SB], F32, tag="ps1",
                                            name=f"ps1_{it}_{pj}_{nb}")
                                   for pj, nb in items]
                            for kc in range(KC):
                                for t, (pj, nb) in zip(pss, items):
                                    nc.tensor.matmul(
                                        t[:],
                                        W_OF[pj][:, kc * P:(kc + 1) * P],
                                        xts0[(kc, nb)],
                                        start=(kc == 0), stop=(kc == KC - 1))
                            for t, (pj, nb) in zip(pss, items):
                                bc = BIAS_COL[pj]
                                dst = DST_OF[pj][:, nb * SB:(nb + 1) * SB]
                                # Q nb2/nb3 copies both go to ACT: it idles
                                # in the settle window, and pv's bank WAR
                                # waits on exactly these copies
                                on_act = ((pj + nb) % 2 == 1
                                          or (pj, nb) == (1, 3))
                                if not on_act:
                                    nc.vector.tensor_scalar_add(
                                        dst, t[:], bias3[:, bc:bc + 1])
                                else:
                                    nc.scalar.add(dst, t[:],
                                                  bias3[:, bc:bc + 1])
                            # V-natural transposes ride the round stream
                            # (V01 ready after round 0, V23 after round 2)
                            if ri == 1:
                                vnat4(0, ps1, "pt4a", bufs=2)
                                vnat4(4, ps1, "pt4a", bufs=2)
                            elif ri == 2:
                                vnat4(8, ps1, "pt4a", bufs=2)
                                vnat4(12, ps1, "pt4a", bufs=2)

                    def load_wo():
                        # emitted after the x-half1 stream: wo's 8 DMAs
                        # otherwise sit ahead of x1 in the serialized DMA
                        # queue and delay the QKV backfill's inputs
                        for kc in range(KC):
                            if bench:
                                nc.vector.memset(wo_s[kc][:], 0.0)
                            else:
                                nc.sync.dma_start(
                                    wo_s[kc][:], woT[kc * P:(kc + 1) * P, :])
                    if phases < 2:
                        load_wo()
                        continue

                    with (
                        tc.tile_pool(name=f"pvp{it}", bufs=1,
                                     space="PSUM") as pvp,
                        tc.tile_pool(name=f"exps{it}", bufs=7) as exps,
                        tc.tile_pool(name=f"norm{it}", bufs=2) as norm,
                    ):
                        with tc.tile_pool(name=f"ps1b{it}", bufs=1,
                                          space="PSUM") as ps1b:
                            with tc.tile_wait_until(0.013):
                                xts1 = load_half(1, xt_pool)

                            def bf_thunk(pj, nb):
                                def run():
                                    # Q-sh1 lands last: its copies go to
                                    # ACT so DVE is clear for b1's first
                                    # Schraudolph exps
                                    eng = (nc.scalar
                                           if (pj, nb) in ((1, 2), (1, 3))
                                           else nc.vector)
                                    qkv_group(pj, nb, R // 2, xts1, ps1b,
                                              "ps1b", eng)
                                    if pj == 2:
                                        # V-natural transposes for this nb
                                        # ride the backfill stream
                                        vnat4(16 + nb * 4, ps1b, "pt4b")
                                return run

                            bf_q = [bf_thunk(pj, nb) for pj, nb in QKV_ORDER]

                            def b0_filler(j):
                                # one QKV-half1 group every ~5 chunks: the
                                # backfill finishes well before b0 ends so
                                # b1's logits aren't input-gated
                                if j % 5 == 4 and bf_q:
                                    bf_q.pop(0)()

                            with tc.tile_pool(name=f"lga{it}", bufs=1,
                                              space="PSUM") as lga:
                                # attention b0 (2+4 banks) with the QKV
                                # half-1 stream interleaved as PE filler
                                attention_batch(0, pvp, 1, lga, 2, exps,
                                                norm, filler=b0_filler)
                                while bf_q:
                                    bf_q.pop(0)()
                            load_wo()

                        with tc.tile_pool(name=f"lgb{it}", bufs=1,
                                          space="PSUM") as lgb:
                            # b1 has no QKV backfill: a 3-deep lg ring (6
                            # banks, using ps1b's freed pair) lets PE run
                            # logits ~2 chunks ahead of the exp chain
                            attention_batch(1, pvp, 1, lgb, 3, exps, norm)

                        if phases < 3:
                            continue
                        # projection PSUM reuses ps1b's 2 banks (free since
                        # mid-attention) so these matmuls can run inside the
                        # final AllToAll window
                        with (
                            tc.tile_pool(name=f"ps4{it}", bufs=2,
                                         space="PSUM") as ps4,
                            tc.tile_pool(name=f"proj{it}", bufs=1) as proj,
                            tc.tile_pool(name=f"outs{it}", bufs=4) as outs,
                        ):
                            rh0 = gather_rh(0, proj)
                            rh1 = gather_rh(1, proj)
                            with tc.tile_wait_until(0.145):
                                proj_batch(0, rh0, ps4, outs)
                            # dependency-free matmuls bridge the final
                            # AllToAll window so the tensor engine's p-state
                            # ramp stays hot for batch-1's projection
                            with tc.tile_wait_until(0.150):
                                for wm in range(110):
                                    wt = ps4.tile([P, 2 * CW], F32,
                                                  tag="ps4",
                                                  name=f"warm_{it}_{wm}")
                                    nc.tensor.matmul(
                                        wt[:], w_k[:, 0:P],
                                        attnT[:, 0:2 * CW],
                                        start=True, stop=True)
                            with tc.tile_wait_until(0.155):
                                proj_batch(1, rh1, ps4, outs)

    nc.compile()
    return nc


def _get_program(n_iters=1, phases=3, bench=False):
    key = (n_iters, phases, bench)
    if key not in _CACHE:
        _CACHE[key] = _build(n_iters, phases, bench)
    return _CACHE[key]


def _w_swizzle(w, sl):
    # device tile layout [P, KC*P]: tile[p, kc*P + c] = w[sl][c, kc*P + p]
    wT = np.asarray(w, np.float32)[sl, :].T.astype(np.float16)  # [D, DL]
    return np.ascontiguousarray(
        wT.reshape(KC, P, DL).transpose(1, 0, 2).reshape(P, D))


def _in_maps(x, wq, bq, wk, bk, wv, bv, wo, bo):
    x = np.asarray(x, np.float32)
    xT = np.ascontiguousarray(x.reshape(R, D).T.astype(np.float16))
    woT = np.ascontiguousarray(
        np.asarray(wo, np.float32).T.astype(np.float16))
    bo_t = np.ascontiguousarray(
        np.asarray(bo, np.float32).reshape(NCORES, P).T)
    maps = []
    for i in range(NCORES):
        sl = slice(i * DL, (i + 1) * DL)
        maps.append({
            "xT": xT,
            "wqS": _w_swizzle(wq, sl),
            "wkS": _w_swizzle(wk, sl),
            "wvS": _w_swizzle(wv, sl),
            "woT": woT,
            "bqkv": np.ascontiguousarray(np.stack(
                [np.asarray(bq, np.float32)[sl],
                 np.asarray(bk, np.float32)[sl],
                 np.asarray(bv, np.float32)[sl]], axis=1)),
            "bo_t": bo_t,
        })
    return maps


def kernel(x, wq, bq, wk, bk, wv, bv, wo, bo, **_):
    nc = _get_program()
    res = run_bass_kernel_spmd(nc, _in_maps(x, wq, bq, wk, bk, wv, bv, wo, bo),
                               list(range(NCORES)))
    # core j holds, for each batch b, output columns
    # [b*2048 + j*256, b*2048 + (j+1)*256) of out.T
    CW = RSL // 2
    outT = np.empty((D, R), np.float32)
    for j in range(NCORES):
        o = res.results[j]["out"]
        for b in range(B):
            outT[:, b * S + j * CW:(b * S) + (j + 1) * CW] = \
                o[:, b * CW:(b + 1) * CW]
    return np.ascontiguousarray(outT.T).reshape(B, S, D)

